# revision 26
# baseline (speedup 1.0000x reference)
"""Causal self-attention Trainium2 kernel (B=8, T=1024, C=768, H=12 heads).

Strategy: data-parallel over batch — one batch element per NeuronCore (8 cores).
Per core, everything is computed in a "transposed" layout so that no on-device
transposes are needed:

  qT, kT  [C, T]   = w_attn_{q,k}.T @ x.T          (x.T supplied by host)
  v_aug   [T, 781] = x @ [w_attn_v | 0]  (+ ones column per head, stride 65)
  sT_h    [Tk, Tq] = kT_h.T-slices @ qT_h          (keys on partitions)
  eT      = exp(sT / 8) with causal mask (memset + triangular multiplicative)
  yT_aug  [65, Tq] = v_aug_h.T @ eT                (row 64 = softmax row-sums)
  yT_norm = yT * broadcast(1/sums)                 (broadcast via one-hot matmul)
  out     [T, C]   = yT_norm.T-slices @ w_proj     (DMA PSUM -> DRAM)

All matmuls run as float32r (reduced-precision fp32 PE mode, ~1.5e-4 rel err,
4x the fp32 throughput at free-dim >= 256).
"""
import sys

sys.path.insert(0, "/opt/trn_rl_repo")

import numpy as np

import concourse.bacc as bacc
import concourse.tile as tile
import concourse.mybir as mybir
from concourse.bass_utils import run_bass_kernel_spmd

f32 = mybir.dt.float32
f32r = mybir.dt.float32r
EXP = mybir.ActivationFunctionType.Exp

B, T, C = 8, 1024, 768
H, D = 12, 64
DA = D + 1  # head stride in v_aug (extra ones column)
NK = C // 128  # 6 contraction tiles
NT = T // 128  # 8 token tiles
SCALE = 1.0 / np.sqrt(D)


def build():
    nc = bacc.Bacc("TRN2", target_bir_lowering=False, debug=False)
    xT = nc.dram_tensor("xT", [C, T], f32r, kind="ExternalInput")
    wq = nc.dram_tensor("wq", [2 * NK, 128, NK, 128], f32r, kind="ExternalInput")
    wv = nc.dram_tensor("wv", [NK, 128, H * DA], f32r, kind="ExternalInput")
    wp = nc.dram_tensor("wp", [NK, 128, C], f32r, kind="ExternalInput")
    msk = nc.dram_tensor("msk", [128, 128], f32r, kind="ExternalInput")
    onesc = nc.dram_tensor("onesc", [128, H], f32r, kind="ExternalInput")
    sel = nc.dram_tensor("sel", [H, C], f32r, kind="ExternalInput")
    out = nc.dram_tensor("out", [T, C], f32, kind="ExternalOutput")

    with tile.TileContext(nc) as tc:
        with (
            tc.tile_pool(name="const", bufs=1) as const,
            tc.tile_pool(name="wqp", bufs=2) as wqp,
            tc.tile_pool(name="exp", bufs=4) as expp,
            tc.tile_pool(name="psc", bufs=3, space="PSUM") as psc,
            tc.tile_pool(name="psm", bufs=2, space="PSUM") as psm,
        ):
            # ---- resident SBUF tensors ----
            xTall = const.tile([128, NK * T], f32r, tag="xTall")
            xT_t = [xTall[:, i * T:(i + 1) * T] for i in range(NK)]
            wvall = const.tile([128, NK * H * DA], f32r, tag="wvall")
            wv_t = [wvall[:, i * H * DA:(i + 1) * H * DA] for i in range(NK)]
            wpall = const.tile([128, NK * C], f32r, tag="wpall")
            wp_t = [wpall[:, i * C:(i + 1) * C] for i in range(NK)]
            qkT_t = [const.tile([128, T], f32r, name=f"qks{m}", tag=f"qk{m}") for m in range(2 * NK)]
            v_t = [const.tile([128, H * DA], f32r, name=f"vs{t}", tag=f"v{t}") for t in range(NT)]
            yT_t = [const.tile([128, T], f32r, name=f"yTs{i}", tag=f"yT{i}") for i in range(NK)]
            msk_t = const.tile([128, 128], f32r, tag="msk")
            ones_t = const.tile([128, H], f32r, tag="ones")
            sel_t = const.tile([H, C], f32r, tag="sel")
            sums_t = const.tile([H, T], f32, tag="sums")
            rec_t = const.tile([H, 512], f32r, tag="rec")

            # phase-1a inputs first (halves, so the first matmuls start sooner)
            xTd = xT.rearrange("(i p) n -> p i n", p=128)
            xTv = xTall.rearrange("p (i n) -> p i n", n=T)
            nc.sync.dma_start(out=xTv[:, :, 0:512], in_=xTd[:, :, 0:512])

            def qk_tile(m, wq_t=None):
                if wq_t is None:
                    wq_t = wqp.tile([128, NK, 128], f32r, tag="wq", name="wq_t")
                    nc.sync.dma_start(out=wq_t, in_=wq[m, :, :, :])
                ps = psc.tile([128, 1024], f32, tag="sc", name="psqk")
                for qc in range(2):
                    for kk in range(NK):
                        nc.tensor.matmul(
                            ps[:, qc * 512:(qc + 1) * 512],
                            wq_t[:, kk, :],
                            xT_t[kk][:, qc * 512:(qc + 1) * 512],
                            start=(kk == 0),
                            stop=(kk == NK - 1),
                        )
                nc.vector.tensor_copy(qkT_t[m], ps)

            def v_tile(t):
                ps = psc.tile([128, 1024], f32, tag="sc", name="psv")
                for n0, nw in ((0, 512), (512, H * DA - 512)):
                    for kk in range(NK):
                        nc.tensor.matmul(
                            ps[:, n0:n0 + nw],
                            xT_t[kk][:, t * 128:(t + 1) * 128],
                            wv_t[kk][:, n0:n0 + nw],
                            start=(kk == 0),
                            stop=(kk == NK - 1),
                        )
                nc.vector.tensor_copy(v_t[t], ps[:, :H * DA])
                ones_ap = v_t[t].rearrange("p (h e) -> p h e", e=DA)[:, :, D]
                nc.vector.tensor_copy(ones_ap, ones_t)

            def attention(hp, qc):
                qs = slice(qc * 512, (qc + 1) * 512)
                nkt = 4 * (qc + 1)
                qT = qkT_t[hp]
                kT = qkT_t[NK + hp]
                ypA = psm.tile([128, 512], f32, tag="mm", name="ypA")
                ypB = psm.tile([128, 512], f32, tag="mm", name="ypB")
                exs = {}
                # software pipeline: attv trails scores/exp by two kt
                LAG = 2
                for kt in range(nkt + LAG):
                    if kt < nkt:
                        ks = slice(kt * 128, (kt + 1) * 128)
                        pos = max(kt * 128 - qc * 512, 0)  # first visible column
                        nv = 512 - pos
                        qv = slice(qc * 512 + pos, (qc + 1) * 512)
                        sp = psc.tile([128, 1024], f32, tag="sc", name="sp")
                        nc.tensor.matmul(
                            sp[:, pos:512], kT[0:64, ks], qT[0:64, qv],
                            start=True, stop=True,
                        )
                        nc.tensor.matmul(
                            sp[:, 512 + pos:1024], kT[64:128, ks], qT[64:128, qv],
                            start=True, stop=True,
                        )
                        ex = expp.tile([128, 1024], f32r, tag="ex", bufs=4, name="ex")
                        if pos == 0:
                            nc.scalar.activation(ex, sp, EXP, scale=float(SCALE))
                        else:
                            exv = ex.rearrange("p (i n) -> p i n", i=2)[:, :, pos:512]
                            spv = sp.rearrange("p (i n) -> p i n", i=2)[:, :, pos:512]
                            nc.scalar.activation(exv, spv, EXP, scale=float(SCALE))
                        if kt * 128 >= qc * 512:  # diagonal tile: triangular mask
                            exd = ex.rearrange("p (i n) -> p i n", i=2)[:, :, pos:pos + 128]
                            mkd = msk_t[:, None, :].broadcast(1, 2) if False else None
                            nc.vector.tensor_mul(exd[:, 0, :], exd[:, 0, :], msk_t)
                            nc.vector.tensor_mul(exd[:, 1, :], exd[:, 1, :], msk_t)
                        exs[kt] = (ex, pos)
                    if kt >= LAG:
                        pk = kt - LAG
                        exp_, ppos = exs.pop(pk)
                        for h, yp, half in ((2 * hp, ypA, 0), (2 * hp + 1, ypB, 1)):
                            nc.tensor.matmul(
                                yp[:DA, ppos:512],
                                v_t[pk][:, h * DA:(h + 1) * DA],
                                exp_[:, half * 512 + ppos:(half + 1) * 512],
                                start=(pk == 0), stop=(pk == nkt - 1),
                            )
                for h, yp, off in ((2 * hp, ypA, 0), (2 * hp + 1, ypB, 64)):
                    stage = expp.tile([DA, 512], f32r, tag="ystage", bufs=2, name="stage")
                    nc.vector.tensor_copy(stage, yp[:DA, :])
                    nc.sync.dma_start(out=yT_t[hp][off:off + 64, qs], in_=stage[:D, :])
                    nc.sync.dma_start(
                        out=sums_t[h:h + 1, qs], in_=stage[D:DA, :].bitcast(f32)
                    )

            def normalize(qc):
                qs = slice(qc * 512, (qc + 1) * 512)
                nc.vector.reciprocal_approx_fast(sums_t[:, qs], sums_t[:, qs])
                with nc.allow_low_precision(reason="f32r recip feeds f32r matmul"):
                    nc.vector.tensor_copy(rec_t, sums_t[:, qs])
                for hp in range(NK):
                    bc = psc.tile([128, 512], f32, tag="sc", name="bc")
                    nc.tensor.matmul(
                        bc, sel_t[:, hp * 128:(hp + 1) * 128], rec_t,
                        start=True, stop=True,
                    )
                    nc.vector.tensor_mul(yT_t[hp][:, qs], yT_t[hp][:, qs], bc.bitcast(f32r))

            def project(qc):
                for t in range(qc * 4, qc * 4 + 4):
                    pp = psc.tile([128, 1024], f32, tag="sc", name="pp")
                    for n0, nw in ((0, 512), (512, 256)):
                        for kk in range(NK):
                            nc.tensor.matmul(
                                pp[:, n0:n0 + nw],
                                yT_t[kk][:, t * 128:(t + 1) * 128],
                                wp_t[kk][:, n0:n0 + nw],
                                start=(kk == 0),
                                stop=(kk == NK - 1),
                            )
                    ostage = expp.tile([128, C], f32, tag="ostage", bufs=2, name="ostage")
                    nc.scalar.copy(ostage, pp[:, :C])
                    nc.sync.dma_start(out=out[t * 128:(t + 1) * 128, :], in_=ostage)

            # ---- interleaved schedule ----
            wq0 = wqp.tile([128, NK, 128], f32r, tag="wq", name="wq0")
            nc.sync.dma_start(out=wq0, in_=wq[0, :, :, :])
            wq6 = wqp.tile([128, NK, 128], f32r, tag="wq", name="wq6")
            nc.scalar.dma_start(out=wq6, in_=wq[6, :, :, :])
            nc.gpsimd.dma_start(out=xTv[:, :, 512:1024], in_=xTd[:, :, 512:1024])
            for hp in range(NK):
                qk_tile(hp, wq0 if hp == 0 else None)
                qk_tile(NK + hp, wq6 if hp == 0 else None)
                if hp == 0:
                    nc.gpsimd.dma_start(
                        out=wvall.rearrange("p (i n) -> p i n", i=NK),
                        in_=wv.rearrange("i p n -> p i n"),
                    )
                    nc.gpsimd.dma_start(out=ones_t, in_=onesc[:, :])
                    nc.gpsimd.dma_start(out=msk_t, in_=msk[:, :])
                    nc.gpsimd.dma_start(out=sel_t, in_=sel[:, :])
                    for t in range(4):
                        v_tile(t)
                else:
                    attention(hp - 1, 0)
                if hp == 2:
                    nc.gpsimd.dma_start(
                        out=wpall.rearrange("p (i n) -> p i n", i=NK),
                        in_=wp.rearrange("i p n -> p i n"),
                    )
                if hp == 5:
                    for t in range(4, NT):
                        v_tile(t)
            attention(5, 0)
            attention(0, 1)
            attention(1, 1)
            normalize(0)
            project(0)
            for hp in range(2, NK):
                attention(hp, 1)
            attention(0, 1) if False else None
            normalize(1)
            project(1)

    nc.compile()
    return nc


_nc = None


def _get_nc():
    global _nc
    if _nc is None:
        _nc = build()
    return _nc


def _host_prep(w_attn, w_proj):
    wq = np.ascontiguousarray(
        w_attn[:, :2 * C].reshape(NK, 128, 2 * NK, 128).transpose(2, 1, 0, 3)
    )
    wv_aug = np.zeros((C, H, DA), np.float32)
    wv_aug[:, :, :D] = w_attn[:, 2 * C:].reshape(C, H, D)
    wv = np.ascontiguousarray(wv_aug.reshape(NK, 128, H * DA))
    wp = np.ascontiguousarray(w_proj.reshape(NK, 128, C))
    msk = np.triu(np.ones((128, 128), np.float32))
    onesc = np.ones((128, H), np.float32)
    sel = np.zeros((H, C), np.float32)
    for p in range(C):
        sel[2 * (p // 128) + (p % 128) // 64, p] = 1.0
    return wq, wv, wp, msk, onesc, sel


def kernel(x, w_attn, w_proj):
    x = np.asarray(x, dtype=np.float32)
    w_attn = np.asarray(w_attn, dtype=np.float32)
    w_proj = np.asarray(w_proj, dtype=np.float32)
    wq, wv, wp, msk, onesc, sel = _host_prep(w_attn, w_proj)
    in_maps = [
        {
            "xT": np.ascontiguousarray(x[b].T),
            "wq": wq,
            "wv": wv,
            "wp": wp,
            "msk": msk,
            "onesc": onesc,
            "sel": sel,
        }
        for b in range(B)
    ]
    res = run_bass_kernel_spmd(_get_nc(), in_maps, list(range(B)))
    return np.stack([res.results[b]["out"] for b in range(B)], axis=0)


# revision 27
# speedup vs baseline: 1.0403x; 1.0403x over previous
"""Causal self-attention Trainium2 kernel (B=8, T=1024, C=768, H=12 heads).

Strategy: data-parallel over batch — one batch element per NeuronCore (8 cores).
Per core, everything is computed in a "transposed" layout so that no on-device
transposes are needed:

  qT, kT  [C, T]   = w_attn_{q,k}.T @ x.T          (x.T supplied by host)
  v_aug   [T, 781] = x @ [w_attn_v | 0]  (+ ones column per head, stride 65)
  sT_h    [Tk, Tq] = kT_h.T-slices @ qT_h          (keys on partitions)
  eT      = exp(sT / 8) with causal mask (memset + triangular multiplicative)
  yT_aug  [65, Tq] = v_aug_h.T @ eT                (row 64 = softmax row-sums)
  yT_norm = yT * broadcast(1/sums)                 (broadcast via one-hot matmul)
  out     [T, C]   = yT_norm.T-slices @ w_proj     (DMA PSUM -> DRAM)

All matmuls run as float32r (reduced-precision fp32 PE mode, ~1.5e-4 rel err,
4x the fp32 throughput at free-dim >= 256).
"""
import sys

sys.path.insert(0, "/opt/trn_rl_repo")

import numpy as np

import concourse.bacc as bacc
import concourse.tile as tile
import concourse.mybir as mybir
from concourse.bass_utils import run_bass_kernel_spmd

f32 = mybir.dt.float32
f32r = mybir.dt.float32r
EXP = mybir.ActivationFunctionType.Exp

B, T, C = 8, 1024, 768
H, D = 12, 64
DA = D + 1  # head stride in v_aug (extra ones column)
NK = C // 128  # 6 contraction tiles
NT = T // 128  # 8 token tiles
SCALE = 1.0 / np.sqrt(D)


def build():
    nc = bacc.Bacc("TRN2", target_bir_lowering=False, debug=False)
    xT = nc.dram_tensor("xT", [C, T], f32r, kind="ExternalInput")
    wq = nc.dram_tensor("wq", [2 * NK, 128, NK, 128], f32r, kind="ExternalInput")
    wv = nc.dram_tensor("wv", [NK, 128, H * DA], f32r, kind="ExternalInput")
    wp = nc.dram_tensor("wp", [NK, 128, C], f32r, kind="ExternalInput")
    msk = nc.dram_tensor("msk", [128, 128], f32r, kind="ExternalInput")
    onesc = nc.dram_tensor("onesc", [128, H], f32r, kind="ExternalInput")
    sel = nc.dram_tensor("sel", [H, C], f32r, kind="ExternalInput")
    out = nc.dram_tensor("out", [T, C], f32, kind="ExternalOutput")

    with tile.TileContext(nc) as tc:
        with (
            tc.tile_pool(name="const", bufs=1) as const,
            tc.tile_pool(name="wqp", bufs=2) as wqp,
            tc.tile_pool(name="exp", bufs=4) as expp,
            tc.tile_pool(name="psc", bufs=3, space="PSUM") as psc,
            tc.tile_pool(name="psm", bufs=2, space="PSUM") as psm,
        ):
            # ---- resident SBUF tensors ----
            xTall = const.tile([128, NK * T], f32r, tag="xTall")
            xT_t = [xTall[:, i * T:(i + 1) * T] for i in range(NK)]
            wvall = const.tile([128, NK * H * DA], f32r, tag="wvall")
            wv_t = [wvall[:, i * H * DA:(i + 1) * H * DA] for i in range(NK)]
            wpall = const.tile([128, NK * C], f32r, tag="wpall")
            wp_t = [wpall[:, i * C:(i + 1) * C] for i in range(NK)]
            qkT_t = [const.tile([128, T], f32r, name=f"qks{m}", tag=f"qk{m}") for m in range(2 * NK)]
            v_t = [const.tile([128, H * DA], f32r, name=f"vs{t}", tag=f"v{t}") for t in range(NT)]
            yT_t = [const.tile([128, T], f32r, name=f"yTs{i}", tag=f"yT{i}") for i in range(NK)]
            msk_t = const.tile([128, 128], f32r, tag="msk")
            ones_t = const.tile([128, H], f32r, tag="ones")
            sel_t = const.tile([H, C], f32r, tag="sel")
            sums_t = const.tile([H, T], f32, tag="sums")
            rec_t = const.tile([H, 512], f32r, tag="rec")

            # phase-1a inputs first (halves, so the first matmuls start sooner)
            xTd = xT.rearrange("(i p) n -> p i n", p=128)
            xTv = xTall.rearrange("p (i n) -> p i n", n=T)
            nc.sync.dma_start(out=xTv[:, :, 0:512], in_=xTd[:, :, 0:512])

            def qk_tile(m, wq_t=None):
                if wq_t is None:
                    wq_t = wqp.tile([128, NK, 128], f32r, tag="wq", name="wq_t")
                    nc.sync.dma_start(out=wq_t, in_=wq[m, :, :, :])
                ps = psc.tile([128, 1024], f32, tag="sc", name="psqk")
                for qc in range(2):
                    for kk in range(NK):
                        nc.tensor.matmul(
                            ps[:, qc * 512:(qc + 1) * 512],
                            wq_t[:, kk, :],
                            xT_t[kk][:, qc * 512:(qc + 1) * 512],
                            start=(kk == 0),
                            stop=(kk == NK - 1),
                        )
                nc.vector.tensor_copy(qkT_t[m], ps)

            def v_tile(t):
                ps = psc.tile([128, 1024], f32, tag="sc", name="psv")
                for n0, nw in ((0, 512), (512, H * DA - 512)):
                    for kk in range(NK):
                        nc.tensor.matmul(
                            ps[:, n0:n0 + nw],
                            xT_t[kk][:, t * 128:(t + 1) * 128],
                            wv_t[kk][:, n0:n0 + nw],
                            start=(kk == 0),
                            stop=(kk == NK - 1),
                        )
                nc.vector.tensor_copy(v_t[t], ps[:, :H * DA])
                ones_ap = v_t[t].rearrange("p (h e) -> p h e", e=DA)[:, :, D]
                nc.vector.tensor_copy(ones_ap, ones_t)

            def attention(hp, qc):
                qs = slice(qc * 512, (qc + 1) * 512)
                nkt = 4 * (qc + 1)
                qT = qkT_t[hp]
                kT = qkT_t[NK + hp]
                ypA = psm.tile([128, 512], f32, tag="mm", name="ypA")
                ypB = psm.tile([128, 512], f32, tag="mm", name="ypB")
                exs = {}
                # software pipeline: attv trails scores/exp by two kt
                LAG = 2
                for kt in range(nkt + LAG):
                    if kt < nkt:
                        ks = slice(kt * 128, (kt + 1) * 128)
                        pos = max(kt * 128 - qc * 512, 0)  # first visible column
                        nv = 512 - pos
                        qv = slice(qc * 512 + pos, (qc + 1) * 512)
                        sp = psc.tile([128, 1024], f32, tag="sc", name="sp")
                        nc.tensor.matmul(
                            sp[:, pos:512], kT[0:64, ks], qT[0:64, qv],
                            start=True, stop=True,
                        )
                        nc.tensor.matmul(
                            sp[:, 512 + pos:1024], kT[64:128, ks], qT[64:128, qv],
                            start=True, stop=True,
                        )
                        ex = expp.tile([128, 1024], f32r, tag="ex", bufs=4, name="ex")
                        if pos == 0:
                            nc.scalar.activation(ex, sp, EXP, scale=float(SCALE))
                        else:
                            exv = ex.rearrange("p (i n) -> p i n", i=2)[:, :, pos:512]
                            spv = sp.rearrange("p (i n) -> p i n", i=2)[:, :, pos:512]
                            nc.scalar.activation(exv, spv, EXP, scale=float(SCALE))
                        if kt * 128 >= qc * 512:  # diagonal tile: triangular mask
                            exd = ex.rearrange("p (i n) -> p i n", i=2)[:, :, pos:pos + 128]
                            mkd = msk_t[:, None, :].broadcast(1, 2) if False else None
                            nc.vector.tensor_mul(exd[:, 0, :], exd[:, 0, :], msk_t)
                            nc.vector.tensor_mul(exd[:, 1, :], exd[:, 1, :], msk_t)
                        exs[kt] = (ex, pos)
                    if kt >= LAG:
                        pk = kt - LAG
                        exp_, ppos = exs.pop(pk)
                        for h, yp, half in ((2 * hp, ypA, 0), (2 * hp + 1, ypB, 1)):
                            nc.tensor.matmul(
                                yp[:DA, ppos:512],
                                v_t[pk][:, h * DA:(h + 1) * DA],
                                exp_[:, half * 512 + ppos:(half + 1) * 512],
                                start=(pk == 0), stop=(pk == nkt - 1),
                            )
                for h, yp, off in ((2 * hp, ypA, 0), (2 * hp + 1, ypB, 64)):
                    stage = expp.tile([DA, 512], f32r, tag="ystage", bufs=2, name="stage")
                    nc.vector.tensor_copy(stage, yp[:DA, :])
                    nc.sync.dma_start(out=yT_t[hp][off:off + 64, qs], in_=stage[:D, :])
                    nc.sync.dma_start(
                        out=sums_t[h:h + 1, qs], in_=stage[D:DA, :].bitcast(f32)
                    )

            def normalize(qc):
                qs = slice(qc * 512, (qc + 1) * 512)
                nc.vector.reciprocal_approx_fast(sums_t[:, qs], sums_t[:, qs])
                with nc.allow_low_precision(reason="f32r recip feeds f32r matmul"):
                    nc.vector.tensor_copy(rec_t, sums_t[:, qs])
                for hp in range(NK):
                    bc = psc.tile([128, 512], f32, tag="sc", name="bc")
                    nc.tensor.matmul(
                        bc, sel_t[:, hp * 128:(hp + 1) * 128], rec_t,
                        start=True, stop=True,
                    )
                    nc.vector.tensor_mul(yT_t[hp][:, qs], yT_t[hp][:, qs], bc.bitcast(f32r))

            def project(ts_range):
                for t in ts_range:
                    pp = psc.tile([128, 1024], f32, tag="sc", name="pp")
                    for n0, nw in ((0, 512), (512, 256)):
                        for kk in range(NK):
                            nc.tensor.matmul(
                                pp[:, n0:n0 + nw],
                                yT_t[kk][:, t * 128:(t + 1) * 128],
                                wp_t[kk][:, n0:n0 + nw],
                                start=(kk == 0),
                                stop=(kk == NK - 1),
                            )
                    ostage = expp.tile([128, C], f32, tag="ostage", bufs=2, name="ostage")
                    nc.scalar.copy(ostage, pp[:, :C])
                    nc.sync.dma_start(out=out[t * 128:(t + 1) * 128, :], in_=ostage)

            # ---- interleaved schedule ----
            wq0 = wqp.tile([128, NK, 128], f32r, tag="wq", name="wq0")
            nc.sync.dma_start(out=wq0, in_=wq[0, :, :, :])
            wq6 = wqp.tile([128, NK, 128], f32r, tag="wq", name="wq6")
            nc.sync.dma_start(out=wq6, in_=wq[6, :, :, :])
            nc.sync.dma_start(out=xTv[:, :, 512:1024], in_=xTd[:, :, 512:1024])
            for hp in range(NK):
                qk_tile(hp, wq0 if hp == 0 else None)
                qk_tile(NK + hp, wq6 if hp == 0 else None)
                if hp == 0:
                    nc.sync.dma_start(
                        out=wvall.rearrange("p (i n) -> p i n", i=NK),
                        in_=wv.rearrange("i p n -> p i n"),
                    )
                    nc.sync.dma_start(out=ones_t, in_=onesc[:, :])
                    nc.sync.dma_start(out=msk_t, in_=msk[:, :])
                    nc.sync.dma_start(out=sel_t, in_=sel[:, :])
                    for t in range(4):
                        v_tile(t)
                else:
                    attention(hp - 1, 0)
                if hp == 2:
                    nc.sync.dma_start(
                        out=wpall.rearrange("p (i n) -> p i n", i=NK),
                        in_=wp.rearrange("i p n -> p i n"),
                    )
                if hp == 5:
                    for t in range(4, NT):
                        v_tile(t)
            attention(5, 0)
            attention(0, 1)
            attention(1, 1)
            normalize(0)
            project(range(0, 2))
            for hp in range(2, NK):
                attention(hp, 1)
            project(range(2, 4))
            normalize(1)
            project(range(4, 8))

    nc.compile()
    return nc


_nc = None


def _get_nc():
    global _nc
    if _nc is None:
        _nc = build()
    return _nc


def _host_prep(w_attn, w_proj):
    wq = np.ascontiguousarray(
        w_attn[:, :2 * C].reshape(NK, 128, 2 * NK, 128).transpose(2, 1, 0, 3)
    )
    wv_aug = np.zeros((C, H, DA), np.float32)
    wv_aug[:, :, :D] = w_attn[:, 2 * C:].reshape(C, H, D)
    wv = np.ascontiguousarray(wv_aug.reshape(NK, 128, H * DA))
    wp = np.ascontiguousarray(w_proj.reshape(NK, 128, C))
    msk = np.triu(np.ones((128, 128), np.float32))
    onesc = np.ones((128, H), np.float32)
    sel = np.zeros((H, C), np.float32)
    for p in range(C):
        sel[2 * (p // 128) + (p % 128) // 64, p] = 1.0
    return wq, wv, wp, msk, onesc, sel


def kernel(x, w_attn, w_proj):
    x = np.asarray(x, dtype=np.float32)
    w_attn = np.asarray(w_attn, dtype=np.float32)
    w_proj = np.asarray(w_proj, dtype=np.float32)
    wq, wv, wp, msk, onesc, sel = _host_prep(w_attn, w_proj)
    in_maps = [
        {
            "xT": np.ascontiguousarray(x[b].T),
            "wq": wq,
            "wv": wv,
            "wp": wp,
            "msk": msk,
            "onesc": onesc,
            "sel": sel,
        }
        for b in range(B)
    ]
    res = run_bass_kernel_spmd(_get_nc(), in_maps, list(range(B)))
    return np.stack([res.results[b]["out"] for b in range(B)], axis=0)


# revision 31
# speedup vs baseline: 1.0581x; 1.0171x over previous
"""Causal self-attention Trainium2 kernel (B=8, T=1024, C=768, H=12 heads).

Strategy: data-parallel over batch — one batch element per NeuronCore (8 cores).
Per core, everything is computed in a "transposed" layout so that no on-device
transposes are needed:

  qT, kT  [C, T]   = w_attn_{q,k}.T @ x.T          (x.T supplied by host)
  v_aug   [T, 781] = x @ [w_attn_v | 0]  (+ ones column per head, stride 65)
  sT_h    [Tk, Tq] = kT_h.T-slices @ qT_h          (keys on partitions)
  eT      = exp(sT / 8) with causal mask (memset + triangular multiplicative)
  yT_aug  [65, Tq] = v_aug_h.T @ eT                (row 64 = softmax row-sums)
  yT_norm = yT * broadcast(1/sums)                 (broadcast via one-hot matmul)
  out     [T, C]   = yT_norm.T-slices @ w_proj     (DMA PSUM -> DRAM)

All matmuls run as float32r (reduced-precision fp32 PE mode, ~1.5e-4 rel err,
4x the fp32 throughput at free-dim >= 256).
"""
import sys

sys.path.insert(0, "/opt/trn_rl_repo")

import numpy as np

import concourse.bass as bass
import concourse.bacc as bacc
import concourse.tile as tile
import concourse.mybir as mybir
from concourse.bass_utils import run_bass_kernel_spmd

f32 = mybir.dt.float32
f32r = mybir.dt.float32r
EXP = mybir.ActivationFunctionType.Exp

B, T, C = 8, 1024, 768
H, D = 12, 64
DA = D + 1  # head stride in v_aug (extra ones column)
NK = C // 128  # 6 contraction tiles
NT = T // 128  # 8 token tiles
SCALE = 1.0 / np.sqrt(D)


def build():
    nc = bacc.Bacc("TRN2", target_bir_lowering=False, debug=False)
    xT = nc.dram_tensor("xT", [C, T], f32r, kind="ExternalInput")
    wq = nc.dram_tensor("wq", [2 * NK, 128, NK, 128], f32r, kind="ExternalInput")
    wv = nc.dram_tensor("wv", [NK, 128, H * DA], f32r, kind="ExternalInput")
    wp = nc.dram_tensor("wp", [NK, 128, C], f32r, kind="ExternalInput")
    msk = nc.dram_tensor("msk", [128, 128], f32r, kind="ExternalInput")
    onesc = nc.dram_tensor("onesc", [128, H], f32r, kind="ExternalInput")
    sel = nc.dram_tensor("sel", [H, C], f32r, kind="ExternalInput")
    out = nc.dram_tensor("out", [T, C], f32, kind="ExternalOutput")

    with tile.TileContext(nc) as tc:
        with (
            tc.tile_pool(name="const", bufs=1) as const,
            tc.tile_pool(name="wqp", bufs=2) as wqp,
            tc.tile_pool(name="exp", bufs=4) as expp,
            tc.tile_pool(name="psc", bufs=3, space="PSUM") as psc,
            tc.tile_pool(name="psm", bufs=2, space="PSUM") as psm,
        ):
            # ---- resident SBUF tensors ----
            xTall = const.tile([128, NK * T], f32r, tag="xTall")
            xT_t = [xTall[:, i * T:(i + 1) * T] for i in range(NK)]
            wvall = const.tile([128, NK * H * DA], f32r, tag="wvall")
            wv_t = [wvall[:, i * H * DA:(i + 1) * H * DA] for i in range(NK)]
            wpall = const.tile([128, NK * C], f32r, tag="wpall")
            wp_t = [wpall[:, i * C:(i + 1) * C] for i in range(NK)]
            qkT_t = [const.tile([128, T], f32r, name=f"qks{m}", tag=f"qk{m}") for m in range(2 * NK)]
            v_t = [const.tile([128, H * DA], f32r, name=f"vs{t}", tag=f"v{t}") for t in range(NT)]
            yT_t = [const.tile([128, T], f32r, name=f"yTs{i}", tag=f"yT{i}") for i in range(NK)]
            msk_t = const.tile([128, 128], f32r, tag="msk")
            ones_t = const.tile([128, H], f32r, tag="ones")
            sel_t = const.tile([H, C], f32r, tag="sel")
            sums_t = const.tile([H, T], f32, tag="sums")
            rec_t = const.tile([H, 512], f32r, tag="rec")

            # phase-1a inputs first (halves, so the first matmuls start sooner)
            xTd = xT.rearrange("(i p) n -> p i n", p=128)
            xTv = xTall.rearrange("p (i n) -> p i n", n=T)

            def qk_tile(m, wq_t=None):
                if wq_t is None:
                    wq_t = wqp.tile([128, NK, 128], f32r, tag="wq", name="wq_t")
                    nc.sync.dma_start(out=wq_t, in_=wq[m, :, :, :])
                ps = psc.tile([128, 1024], f32, tag="sc", name="psqk")
                for qc in range(2):
                    for kk in range(NK):
                        nc.tensor.matmul(
                            ps[:, qc * 512:(qc + 1) * 512],
                            wq_t[:, kk, :],
                            xT_t[kk][:, qc * 512:(qc + 1) * 512],
                            start=(kk == 0),
                            stop=(kk == NK - 1),
                        )
                nc.vector.tensor_copy(qkT_t[m], ps)

            def v_tile(t):
                ps = psc.tile([128, 1024], f32, tag="sc", name="psv")
                for n0, nw in ((0, 512), (512, H * DA - 512)):
                    for kk in range(NK):
                        nc.tensor.matmul(
                            ps[:, n0:n0 + nw],
                            xT_t[kk][:, t * 128:(t + 1) * 128],
                            wv_t[kk][:, n0:n0 + nw],
                            start=(kk == 0),
                            stop=(kk == NK - 1),
                        )
                nc.vector.tensor_copy(v_t[t], ps[:, :H * DA])
                ones_ap = v_t[t].rearrange("p (h e) -> p h e", e=DA)[:, :, D]
                nc.vector.tensor_copy(ones_ap, ones_t)

            def attention(hp, qc):
                qs = slice(qc * 512, (qc + 1) * 512)
                nkt = 4 * (qc + 1)
                qT = qkT_t[hp]
                kT = qkT_t[NK + hp]
                ypA = psm.tile([128, 512], f32, tag="mm", name="ypA")
                ypB = psm.tile([128, 512], f32, tag="mm", name="ypB")
                exs = {}
                # software pipeline: attv trails scores/exp by two kt
                LAG = 2
                for kt in range(nkt + LAG):
                    if kt < nkt:
                        ks = slice(kt * 128, (kt + 1) * 128)
                        pos = max(kt * 128 - qc * 512, 0)  # first visible column
                        nv = 512 - pos
                        qv = slice(qc * 512 + pos, (qc + 1) * 512)
                        sp = psc.tile([128, 1024], f32, tag="sc", name="sp")
                        nc.tensor.matmul(
                            sp[:, pos:512], kT[0:64, ks], qT[0:64, qv],
                            start=True, stop=True,
                        )
                        nc.tensor.matmul(
                            sp[:, 512 + pos:1024], kT[64:128, ks], qT[64:128, qv],
                            start=True, stop=True,
                        )
                        ex = expp.tile([128, 1024], f32r, tag="ex", bufs=4, name="ex")
                        if pos == 0:
                            nc.scalar.activation(ex, sp, EXP, scale=float(SCALE))
                        else:
                            exv = ex.rearrange("p (i n) -> p i n", i=2)[:, :, pos:512]
                            spv = sp.rearrange("p (i n) -> p i n", i=2)[:, :, pos:512]
                            nc.scalar.activation(exv, spv, EXP, scale=float(SCALE))
                        if kt * 128 >= qc * 512:  # diagonal tile: triangular mask
                            exd = ex.rearrange("p (i n) -> p i n", i=2)[:, :, pos:pos + 128]
                            mkd = msk_t[:, None, :].broadcast(1, 2) if False else None
                            nc.vector.tensor_mul(exd[:, 0, :], exd[:, 0, :], msk_t)
                            nc.vector.tensor_mul(exd[:, 1, :], exd[:, 1, :], msk_t)
                        exs[kt] = (ex, pos)
                    if kt >= LAG:
                        pk = kt - LAG
                        exp_, ppos = exs.pop(pk)
                        for h, yp, half in ((2 * hp, ypA, 0), (2 * hp + 1, ypB, 1)):
                            nc.tensor.matmul(
                                yp[:DA, ppos:512],
                                v_t[pk][:, h * DA:(h + 1) * DA],
                                exp_[:, half * 512 + ppos:(half + 1) * 512],
                                start=(pk == 0), stop=(pk == nkt - 1),
                            )
                for h, yp, off in ((2 * hp, ypA, 0), (2 * hp + 1, ypB, 64)):
                    stage = expp.tile([DA, 512], f32r, tag="ystage", bufs=2, name="stage")
                    nc.vector.tensor_copy(stage, yp[:DA, :])
                    nc.sync.dma_start(out=yT_t[hp][off:off + 64, qs], in_=stage[:D, :])
                    nc.sync.dma_start(
                        out=sums_t[h:h + 1, qs], in_=stage[D:DA, :].bitcast(f32)
                    )

            def normalize(qc):
                qs = slice(qc * 512, (qc + 1) * 512)
                nc.vector.reciprocal_approx_fast(sums_t[:, qs], sums_t[:, qs])
                with nc.allow_low_precision(reason="f32r recip feeds f32r matmul"):
                    nc.vector.tensor_copy(rec_t, sums_t[:, qs])
                for hp in range(NK):
                    bc = psc.tile([128, 512], f32, tag="sc", name="bc")
                    nc.tensor.matmul(
                        bc, sel_t[:, hp * 128:(hp + 1) * 128], rec_t,
                        start=True, stop=True,
                    )
                    nc.vector.tensor_mul(yT_t[hp][:, qs], yT_t[hp][:, qs], bc.bitcast(f32r))

            def project(ts_range):
                for t in ts_range:
                    pp = psc.tile([128, 1024], f32, tag="sc", name="pp")
                    for n0, nw in ((0, 512), (512, 256)):
                        for kk in range(NK):
                            nc.tensor.matmul(
                                pp[:, n0:n0 + nw],
                                yT_t[kk][:, t * 128:(t + 1) * 128],
                                wp_t[kk][:, n0:n0 + nw],
                                start=(kk == 0),
                                stop=(kk == NK - 1),
                            )
                    ostage = expp.tile([128, C], f32, tag="ostage", bufs=2, name="ostage")
                    nc.scalar.copy(ostage, pp[:, :C])
                    nc.sync.dma_start(out=out[t * 128:(t + 1) * 128, :], in_=ostage)

            # ---- interleaved schedule ----
            wq0 = wqp.tile([128, NK, 128], f32r, tag="wq", name="wq0")
            nc.sync.dma_start(out=wq0, in_=wq[0, :, :, :])
            nc.sync.dma_start(out=xTv[:, :, 0:512], in_=xTd[:, :, 0:512])
            nc.sync.dma_start(out=xTv[:, :, 512:1024], in_=xTd[:, :, 512:1024])
            wq6 = wqp.tile([128, NK, 128], f32r, tag="wq", name="wq6")
            nc.sync.dma_start(out=wq6, in_=wq[6, :, :, :])
            for hp in range(NK):
                qk_tile(hp, wq0 if hp == 0 else None)
                qk_tile(NK + hp, wq6 if hp == 0 else None)
                if hp == 0:
                    nc.sync.dma_start(
                        out=wvall.rearrange("p (i n) -> p i n", i=NK),
                        in_=wv.rearrange("i p n -> p i n"),
                    )
                    nc.sync.dma_start(out=ones_t, in_=onesc[:, :])
                    nc.sync.dma_start(out=msk_t, in_=msk[:, :])
                    nc.sync.dma_start(out=sel_t, in_=sel[:, :])
                    for t in range(4):
                        v_tile(t)
                else:
                    attention(hp - 1, 0)
                if hp == 2:
                    nc.sync.dma_start(
                        out=wpall.rearrange("p (i n) -> p i n", i=NK),
                        in_=wp.rearrange("i p n -> p i n"),
                    )
                if hp == 5:
                    for t in range(4, NT):
                        v_tile(t)
            attention(5, 0)
            attention(0, 1)
            attention(1, 1)
            normalize(0)
            project(range(0, 2))
            for hp in range(2, NK):
                attention(hp, 1)
            project(range(2, 4))
            normalize(1)
            project(range(4, 8))

    nc.compile()
    return nc


_nc = None


def _get_nc():
    global _nc
    if _nc is None:
        _nc = build()
    return _nc


def _host_prep(w_attn, w_proj):
    wq = np.ascontiguousarray(
        w_attn[:, :2 * C].reshape(NK, 128, 2 * NK, 128).transpose(2, 1, 0, 3)
    )
    wv_aug = np.zeros((C, H, DA), np.float32)
    wv_aug[:, :, :D] = w_attn[:, 2 * C:].reshape(C, H, D)
    wv = np.ascontiguousarray(wv_aug.reshape(NK, 128, H * DA))
    wp = np.ascontiguousarray(w_proj.reshape(NK, 128, C))
    msk = np.triu(np.ones((128, 128), np.float32))
    onesc = np.ones((128, H), np.float32)
    sel = np.zeros((H, C), np.float32)
    for p in range(C):
        sel[2 * (p // 128) + (p % 128) // 64, p] = 1.0
    return wq, wv, wp, msk, onesc, sel


def kernel(x, w_attn, w_proj):
    x = np.asarray(x, dtype=np.float32)
    w_attn = np.asarray(w_attn, dtype=np.float32)
    w_proj = np.asarray(w_proj, dtype=np.float32)
    wq, wv, wp, msk, onesc, sel = _host_prep(w_attn, w_proj)
    in_maps = [
        {
            "xT": np.ascontiguousarray(x[b].T),
            "wq": wq,
            "wv": wv,
            "wp": wp,
            "msk": msk,
            "onesc": onesc,
            "sel": sel,
        }
        for b in range(B)
    ]
    res = run_bass_kernel_spmd(_get_nc(), in_maps, list(range(B)))
    return np.stack([res.results[b]["out"] for b in range(B)], axis=0)
